# revision 1
# baseline (speedup 1.0000x reference)
"""AttentionPooling (segment softmax pooling) on 8 Trainium2 NeuronCores.

z[b] = sum_i softmax_within_segment(alpha)_i * x_i  for segment b, where
alpha = tanh(x @ W1.T) @ W2.T.

Strategy (data parallel over segments):
- batch is sorted, B = 1024 = 8 * 128, so core c owns segments
  [128c, 128(c+1)) — a contiguous row range of x. No cross-core segments.
- alpha range for this distribution is ~[-3, 3], so exp() without the
  per-segment max subtraction is numerically safe; softmax = e / seg_sum(e)
  and both numerator and denominator are plain segment sums the device
  accumulates in one pass over x.
- Per 128-row tile on device:
    yT   = W1 @ x_tile.T          (PE, bf16, K=256 via 2 chunks)
    th   = tanh(yT)               (ACT, PSUM->SBUF bf16)
    a    = th.T @ W2              (PE -> (128 rows x 1) PSUM)
    e    = exp(a)                 (ACT)
    E    = (iota == colidx) * e   (DVE one-hot(local seg) weighted by e)
    pool += E.T @ [x_tile | 1]    (PE, persistent (128 segs x 257) PSUM)
  pool[:, :256] = sum e*x (numerator), pool[:, 256] = sum e (denominator).
- Host: z = pool[:, :256] / pool[:, 256:257], concat cores.

x is shipped twice in bf16 (row-major for pooling, transposed for the
matmul contraction over D) — 2 bytes * 2 orientations = same HBM traffic
as reading the f32 x once; the kernel is HBM-bandwidth bound.
"""

import numpy as np
import ml_dtypes

import concourse.bacc as bacc
import concourse.mybir as mybir
import concourse.tile as tile
from concourse.bass_utils import run_bass_kernel_spmd

bf16 = ml_dtypes.bfloat16
F32 = mybir.dt.float32
BF16 = mybir.dt.bfloat16
AF = mybir.ActivationFunctionType
ALU = mybir.AluOpType

NCORES = 8
D = 256
H = 128
SEGS_PER_CORE = 128
GT = 16          # tiles per DMA group
QUAD = 4         # tiles per mm1/psum_y batch (N' = 512)

_kernel_cache = {}


def _build_kernel(nt):
    """Build + compile the per-core SPMD kernel for nt 128-row tiles."""
    assert nt % GT == 0 and GT % QUAD == 0
    nc = bacc.Bacc("TRN2", target_bir_lowering=False, debug=False)

    x_nat_d = nc.dram_tensor("x_nat", [128, nt, D + 1], BF16, kind="ExternalInput").ap()
    xt_d = nc.dram_tensor("xT", [128, 2, nt * 128], BF16, kind="ExternalInput").ap()
    ci_d = nc.dram_tensor("colidx", [128, nt], F32, kind="ExternalInput").ap()
    w1t_d = nc.dram_tensor("W1T", [128, 2, H], BF16, kind="ExternalInput").ap()
    w2_d = nc.dram_tensor("W2c", [H, 1], BF16, kind="ExternalInput").ap()
    iota_d = nc.dram_tensor("iota", [128, SEGS_PER_CORE], F32, kind="ExternalInput").ap()
    out_d = nc.dram_tensor("out", [SEGS_PER_CORE, D + 1], F32, kind="ExternalOutput").ap()

    ngroups = nt // GT
    with tile.TileContext(nc) as tc:
        with (
            tc.tile_pool(name="const", bufs=1) as constp,
            tc.tile_pool(name="xn", bufs=3) as xnp,
            tc.tile_pool(name="xt", bufs=3) as xtp,
            tc.tile_pool(name="th", bufs=3) as thp,
            tc.tile_pool(name="ee", bufs=4) as eep,
            tc.tile_pool(name="out", bufs=1) as outp,
            tc.tile_pool(name="psum_y", bufs=2, space="PSUM") as psumy,
            tc.tile_pool(name="psum_al", bufs=2, space="PSUM") as psumal,
            tc.tile_pool(name="psum_acc", bufs=1, space="PSUM") as psumacc,
        ):
            w1t_sb = constp.tile([128, 2, H], BF16)
            nc.default_dma_engine.dma_start(w1t_sb[:], w1t_d[:])
            w2_sb = constp.tile([H, 1], BF16)
            nc.default_dma_engine.dma_start(w2_sb[:], w2_d[:])
            iota_sb = constp.tile([128, SEGS_PER_CORE], F32)
            nc.default_dma_engine.dma_start(iota_sb[:], iota_d[:])
            ci_sb = constp.tile([128, nt], F32)
            nc.default_dma_engine.dma_start(ci_sb[:], ci_d[:])

            pool_ps = psumacc.tile([SEGS_PER_CORE, D + 1], F32)

            for g in range(ngroups):
                xn = xnp.tile([128, GT, D + 1], BF16, tag="xn")
                nc.default_dma_engine.dma_start(xn[:], x_nat_d[:, g * GT:(g + 1) * GT, :])
                xt = xtp.tile([128, 2, GT * 128], BF16, tag="xt")
                nc.default_dma_engine.dma_start(
                    xt[:], xt_d[:, :, g * GT * 128:(g + 1) * GT * 128])

                for q in range(GT // QUAD):
                    y_ps = psumy.tile([128, QUAD * 128], F32, tag="y")
                    nc.tensor.matmul(y_ps[:], w1t_sb[:, 0, :],
                                     xt[:, 0, q * QUAD * 128:(q + 1) * QUAD * 128],
                                     start=True, stop=False)
                    nc.tensor.matmul(y_ps[:], w1t_sb[:, 1, :],
                                     xt[:, 1, q * QUAD * 128:(q + 1) * QUAD * 128],
                                     start=False, stop=True)
                    th = thp.tile([128, QUAD * 128], BF16, tag="th")
                    nc.scalar.activation(th[:], y_ps[:], AF.Tanh)

                    al_ps = psumal.tile([128, QUAD], F32, tag="al")
                    for j in range(QUAD):
                        nc.tensor.matmul(al_ps[:, j:j + 1], th[:, j * 128:(j + 1) * 128],
                                         w2_sb[:], start=True, stop=True)
                    e_sb = eep.tile([128, QUAD], F32, tag="e")
                    nc.scalar.activation(e_sb[:], al_ps[:], AF.Exp)

                    for j in range(QUAD):
                        t = g * GT + q * QUAD + j
                        E = eep.tile([128, SEGS_PER_CORE], BF16, tag="E")
                        nc.vector.tensor_scalar(
                            E[:], iota_sb[:], ci_sb[:, t:t + 1], e_sb[:, j:j + 1],
                            ALU.is_equal, ALU.mult)
                        nc.tensor.matmul(pool_ps[:], E[:], xn[:, q * QUAD + j, :],
                                         start=(t == 0), stop=(t == nt - 1))

            pool_sb = outp.tile([SEGS_PER_CORE, D + 1], F32)
            nc.scalar.activation(pool_sb[:], pool_ps[:], AF.Copy)
            nc.default_dma_engine.dma_start(out_d[:], pool_sb[:])

    nc.compile()
    return nc


def _prep_core(x, batch, r0, r1, seg0, nt):
    """Host-side shard prep for one core: rows [r0, r1) own segments
    [seg0, seg0+128). Returns the per-core input map."""
    rows = r1 - r0
    pad_rows = nt * 128

    xb = np.zeros((pad_rows, D + 1), dtype=bf16)
    xb[:rows, :D] = x[r0:r1].astype(bf16)
    xb[:, D] = bf16(1.0)
    # (128, nt, 257): partition p holds row t*128 + p
    x_nat = np.ascontiguousarray(xb.reshape(nt, 128, D + 1).transpose(1, 0, 2))

    xtb = np.zeros((2, H, pad_rows), dtype=bf16)
    xtb.reshape(D, pad_rows)[:, :rows] = x[r0:r1].astype(bf16).T
    xT = np.ascontiguousarray(xtb.transpose(1, 0, 2))  # (128, 2, pad_rows)

    ci = np.full(pad_rows, -1.0, dtype=np.float32)
    ci[:rows] = (batch[r0:r1] - seg0).astype(np.float32)
    colidx = np.ascontiguousarray(ci.reshape(nt, 128).T)  # (128, nt)

    return {"x_nat": x_nat, "xT": xT, "colidx": colidx}


def kernel(x, batch, W1, W2, B):
    x = np.asarray(x)
    batch = np.asarray(batch)
    W1 = np.asarray(W1)
    W2 = np.asarray(W2)
    B = int(B)
    n = x.shape[0]
    assert B == NCORES * SEGS_PER_CORE

    seg_starts = np.searchsorted(batch, np.arange(0, B + 1, SEGS_PER_CORE))
    seg_starts[0], seg_starts[-1] = 0, n
    nt = int(max(
        -(-(int(seg_starts[c + 1] - seg_starts[c])) // 128) for c in range(NCORES)))
    nt = -(-nt // GT) * GT  # round up to a full DMA group

    if nt not in _kernel_cache:
        _kernel_cache[nt] = _build_kernel(nt)
    nc = _kernel_cache[nt]

    w1t = np.ascontiguousarray(
        W1.T.astype(bf16).reshape(2, H, H).transpose(1, 0, 2))  # (128, 2, H)
    w2c = np.ascontiguousarray(W2.reshape(H, 1).astype(bf16))
    iota = np.broadcast_to(
        np.arange(SEGS_PER_CORE, dtype=np.float32), (128, SEGS_PER_CORE)).copy()

    in_maps = []
    for c in range(NCORES):
        m = _prep_core(x, batch, int(seg_starts[c]), int(seg_starts[c + 1]),
                       c * SEGS_PER_CORE, nt)
        m.update({"W1T": w1t, "W2c": w2c, "iota": iota})
        in_maps.append(m)

    res = run_bass_kernel_spmd(nc, in_maps, core_ids=list(range(NCORES)))

    z = np.empty((B, D), dtype=np.float32)
    for c in range(NCORES):
        out = res.results[c]["out"]
        num = out[:, :D]
        den = out[:, D:D + 1]
        den = np.where(den == 0.0, 1.0, den)
        z[c * SEGS_PER_CORE:(c + 1) * SEGS_PER_CORE] = num / den
    return z
